# revision 1
# baseline (speedup 1.0000x reference)
"""Causal self-attention block (QKV proj + causal MHA + out proj + residual
+ LayerNorm) for B=4, S=2048, HID=1024, 16 heads, on 8 Trainium2 cores.

Sharding: core c handles batch b=c//2 and heads [8h, 8h+8) where h=c%2
(Megatron-style head split within a batch pair). Each core computes its 8
heads' attention and a partial output projection over the full 2048 rows;
the two cores of a batch pair combine partials with pairwise
ReduceScatters (chunked, pipelined with compute), then each core applies
residual + LayerNorm to its quarter-rows and returns [1024, 1024].

Schedule (v2): initial weight/x DMAs are spread across the three
DMA-capable queues (sync/scalar/gpsimd) so the first projections start
~20us earlier. The attention inner loop is software-pipelined (QK runs
two i-blocks ahead of PV) so the PE never queues behind the scalar-engine
exp. Projections for tile t+1 are emitted *before* the out-projection of
tile t, filling the softmax-normalize latency with independent matmuls.
Causal masking and LayerNorm gamma/beta run on the Pool engine; LayerNorm
1/sigma uses exp(-0.5*ln(var+eps)) so the scalar engine never swaps
activation tables (exp and ln share a table set). ReduceScatter partials
travel as bf16, halving the pairwise-collective transfer.
"""

import numpy as np

import concourse.bacc as bacc
import concourse.mybir as mybir
import concourse.tile as tile
from concourse.bass import broadcast_tensor_aps
from concourse.bass_utils import run_bass_kernel_spmd

F32 = mybir.dt.float32
F32R = mybir.dt.float32r
BF16 = mybir.dt.bfloat16
AF = mybir.ActivationFunctionType
OP = mybir.AluOpType

N_CORES = 8
B, S, HID = 4, 2048, 1024
NHC = 8          # heads per core
DH = 64          # head dim
HW = 512         # per-core head width (NHC * DH)
SQT = 512        # sq tile width
NSQT = S // SQT  # 4
NHCH = HID // 128  # 8 hid chunks
SH = S // 2      # rows per core in the epilogue
EPS = 1e-5

_CACHE = {}


def _build():
    nc = bacc.Bacc("TRN2", target_bir_lowering=False, debug=False,
                   num_devices=N_CORES)

    xT = nc.dram_tensor("xT", [HID, S], F32R, kind="ExternalInput").ap()
    xh = nc.dram_tensor("xh", [SH, HID], F32, kind="ExternalInput").ap()
    wqT = nc.dram_tensor("wqT", [HID, HW], F32R, kind="ExternalInput").ap()
    wkT = nc.dram_tensor("wkT", [HID, HW], F32R, kind="ExternalInput").ap()
    wvT = nc.dram_tensor("wvT", [HID, HW], F32R, kind="ExternalInput").ap()
    woT = nc.dram_tensor("woT", [HW, HID], F32R, kind="ExternalInput").ap()
    bq4 = nc.dram_tensor("bq4", [128, 4], F32, kind="ExternalInput").ap()
    bk4 = nc.dram_tensor("bk4", [128, 4], F32, kind="ExternalInput").ap()
    bvb = nc.dram_tensor("bvb", [128, HW], F32, kind="ExternalInput").ap()
    gmb = nc.dram_tensor("gmb", [128, HID], F32, kind="ExternalInput").ap()
    btb = nc.dram_tensor("btb", [128, HID], F32, kind="ExternalInput").ap()
    m128 = nc.dram_tensor("m128", [128, 128], F32, kind="ExternalInput").ap()
    vone = nc.dram_tensor("vone", [128, 8], F32R, kind="ExternalInput").ap()

    out = nc.dram_tensor("out", [SH, HID], F32, kind="ExternalOutput").ap()

    po_d = nc.dram_tensor("po_d", [S, HID], BF16)
    rs_d = nc.dram_tensor("rs_d", [SH, HID], BF16)

    from contextlib import ExitStack
    with tile.TileContext(nc) as tc, ExitStack() as es:
        TP = tc.tile_pool
        cp = es.enter_context(TP(name="consts", bufs=1))
        ktp = es.enter_context(TP(name="kt", bufs=1))
        vtp = es.enter_context(TP(name="vt", bufs=1))
        wop = es.enter_context(TP(name="wo", bufs=1))
        ep = es.enter_context(TP(name="exp", bufs=2))
        atp = es.enter_context(TP(name="att", bufs=1))
        avp = es.enter_context(TP(name="av", bufs=1))
        rp = es.enter_context(TP(name="rcp", bufs=2))
        poep = es.enter_context(TP(name="poe", bufs=1))
        pp = es.enter_context(TP(name="pp", bufs=2, space="PSUM"))
        sp = es.enter_context(TP(name="sp", bufs=2, space="PSUM"))
        app = es.enter_context(TP(name="ap", bufs=1, space="PSUM"))
        wp = es.enter_context(TP(name="wqkv", bufs=1))
        xp = es.enter_context(TP(name="xts", bufs=1))
        qtp = es.enter_context(TP(name="qt", bufs=1))
        lp = es.enter_context(TP(name="ln", bufs=1))
        lsp = es.enter_context(TP(name="lns", bufs=2))

        # ---- constants + weights, spread across the three DMA queues ----
        # sync: wq (Q proj, first need) then wk; scalar: x tile 0 then wv;
        # gpsimd: small consts + wo + gamma/beta.
        wq = [wp.tile([128, HW], F32R, name=f"wq{hh}") for hh in range(NHCH)]
        wk = [wp.tile([128, HW], F32R, name=f"wk{hh}") for hh in range(NHCH)]
        wv = [wp.tile([128, HW], F32R, name=f"wv{hh}") for hh in range(NHCH)]
        for hh in range(NHCH):
            nc.sync.dma_start(wq[hh][:], wqT[128 * hh:128 * (hh + 1), :])
        xts = [xp.tile([128, SQT], F32R, tag=f"xt{hh}", name=f"xt{hh}")
               for hh in range(NHCH)]
        for hh in range(NHCH):
            nc.scalar.dma_start(xts[hh][:], xT[128 * hh:128 * (hh + 1),
                                              0:SQT])
        for hh in range(NHCH):
            nc.sync.dma_start(wk[hh][:], wkT[128 * hh:128 * (hh + 1), :])
        for hh in range(NHCH):
            nc.scalar.dma_start(wv[hh][:], wvT[128 * hh:128 * (hh + 1), :])

        mask = cp.tile([128, 128], F32)
        nc.gpsimd.dma_start(mask[:], m128[:])
        bqs = cp.tile([128, 4], F32)
        nc.gpsimd.dma_start(bqs[:], bq4[:])
        bks = cp.tile([128, 4], F32)
        nc.gpsimd.dma_start(bks[:], bk4[:])
        bvs = cp.tile([128, HW], F32)
        nc.gpsimd.dma_start(bvs[:], bvb[:])
        vos = cp.tile([128, 8], F32R)
        nc.gpsimd.dma_start(vos[:], vone[:])
        magicc = cp.tile([128, 1], mybir.dt.uint32)
        nc.vector.memset(magicc[:], 0x5f3759df)
        mhalf = cp.tile([128, 1], F32)
        nc.vector.memset(mhalf[:], -0.5)
        wot = [wop.tile([128, HID], F32R, name=f"wo{d}") for d in range(4)]
        for d in range(4):
            nc.sync.dma_start(wot[d][:], woT[128 * d:128 * (d + 1), :])
        gms = cp.tile([128, HID], F32)
        nc.sync.dma_start(gms[:], gmb[:])
        bts = cp.tile([128, HID], F32)
        nc.sync.dma_start(bts[:], btb[:])

        kt = [ktp.tile([128, S], F32R, name=f"kt{p}") for p in range(4)]
        vt = [vtp.tile([128, 8, 65], F32R, name=f"vt{i}") for i in range(16)]

        qts = [None] * 4
        at_tiles = [None] * 4

        def emit_A(t):
            """QKV projections for sq tile t (tile 0 x-DMAs already issued)."""
            if t > 0:
                for hh in range(NHCH):
                    nc.sync.dma_start(
                        xts[hh][:], xT[128 * hh:128 * (hh + 1),
                                       SQT * t:SQT * (t + 1)])
            for m in range(4):
                ps = pp.tile([128, SQT], F32, tag="pq")
                for hh in range(NHCH):
                    nc.tensor.matmul(
                        ps[:], wq[hh][:, 128 * m:128 * (m + 1)],
                        xts[hh][:], start=(hh == 0), stop=(hh == NHCH - 1))
                qt_ = qtp.tile([128, SQT], F32R, tag=f"q{m}")
                nc.vector.tensor_scalar_add(qt_[:], ps[:], bqs[:, m:m + 1])
                qts[m] = qt_
            for m in range(4):
                ps = pp.tile([128, SQT], F32, tag="pq")
                for hh in range(NHCH):
                    nc.tensor.matmul(
                        ps[:], wk[hh][:, 128 * m:128 * (m + 1)],
                        xts[hh][:], start=(hh == 0), stop=(hh == NHCH - 1))
                nc.vector.tensor_scalar_add(
                    kt[m][:, SQT * t:SQT * (t + 1)], ps[:], bks[:, m:m + 1])
            for s_ in range(4):
                i = 4 * t + s_
                ps = pp.tile([128, HW], F32, tag="pq")
                for hh in range(NHCH):
                    nc.tensor.matmul(
                        ps[:], xts[hh][:, 128 * s_:128 * (s_ + 1)],
                        wv[hh][:], start=(hh == 0), stop=(hh == NHCH - 1))
                nc.vector.tensor_tensor(
                    vt[i][:, :, 0:64], ps[:], bvs[:], op=OP.add)
                nc.vector.tensor_copy(vt[i][:, :, 64:65], vos[:])

        def emit_QK(p, i, j):
            """Scores for key block i, head pair p, query tile j -> s2."""
            d = i - 4 * j
            lo_qk = min(128 * d, 256) if d >= 0 else 0
            s2 = sp.tile([128, 2 * SQT], F32, tag="s2")
            nc.tensor.matmul(
                s2[:, lo_qk:SQT],
                kt[p][0:64, 128 * i:128 * (i + 1)],
                qts[p][0:64, lo_qk:SQT],
                start=True, stop=True, tile_position=(0, 0))
            nc.tensor.matmul(
                s2[:, SQT + lo_qk:2 * SQT],
                kt[p][64:128, 128 * i:128 * (i + 1)],
                qts[p][64:128, lo_qk:SQT],
                start=True, stop=True, tile_position=(64, 0))
            return s2

        def emit_exp(p, i, j, s2):
            d = i - 4 * j
            lo_qk = min(128 * d, 256) if d >= 0 else 0
            lo = 128 * d if d >= 0 else 0
            e2 = ep.tile([128, 2 * SQT], F32R, tag="e2")
            s2v = s2[:].rearrange("p (a b) -> p a b", a=2)
            e2v = e2[:].rearrange("p (a b) -> p a b", a=2)
            nc.scalar.activation(e2v[:, :, lo_qk:SQT], s2v[:, :, lo_qk:SQT],
                                 AF.Exp, scale=0.125)
            if d >= 0:
                ea = e2v[:, :, lo:lo + 128]
                ma = mask[:].rearrange("p (a b) -> p a b", a=1)
                ea2, ma2 = broadcast_tensor_aps(ea, ma)
                nc.vector.tensor_tensor(ea2, ea2, ma2, op=OP.mult)
            return e2

        def emit_PV(p, i, j, e2, pv2):
            d = i - 4 * j
            lo = 128 * d if d >= 0 else 0
            nc.tensor.matmul(
                pv2[0:65, lo:SQT], vt[i][:, 2 * p, :], e2[:, lo:SQT],
                start=(i == 0), stop=(i == 4 * j + 3))
            nc.tensor.matmul(
                pv2[0:65, SQT + lo:2 * SQT], vt[i][:, 2 * p + 1, :],
                e2[:, SQT + lo:2 * SQT],
                start=(i == 0), stop=(i == 4 * j + 3))

        def emit_B(j, mid_hook=None):
            """Attention for sq tile j, software-pipelined by two i-blocks."""
            ni = 4 * j + 4
            for p in range(4):
                if p == 2 and mid_hook is not None:
                    mid_hook()
                pv2 = app.tile([128, 2 * SQT], F32, tag="pv2")
                s2s = {0: emit_QK(p, 0, j)}
                if ni > 1:
                    s2s[1] = emit_QK(p, 1, j)
                for i in range(ni):
                    e2 = emit_exp(p, i, j, s2s.pop(i))
                    emit_PV(p, i, j, e2, pv2)
                    if i + 2 < ni:
                        s2s[i + 2] = emit_QK(p, i + 2, j)
                # normalize: evac PSUM, recip of sums row, broadcast, scale
                av2 = avp.tile([65, 2 * SQT], F32, tag="av")
                nc.vector.tensor_copy(av2[:], pv2[0:65, :])
                at_ = atp.tile([128, SQT], F32R, tag=f"at{p}")
                for hb in range(2):
                    sm = rp.tile([1, SQT], F32, tag="sm", bufs=1)
                    nc.vector.tensor_copy(
                        sm[:], av2[64:65, SQT * hb:SQT * (hb + 1)])
                    rc = rp.tile([1, SQT], F32, tag="rc", bufs=1)
                    nc.vector.reciprocal_approx_fast(rc[:], sm[:])
                    rb = rp.tile([64, SQT], F32, tag="rb", bufs=1)
                    nc.gpsimd.partition_broadcast(rb[:], rc[:])
                    nc.vector.tensor_mul(
                        at_[64 * hb:64 * (hb + 1), :],
                        av2[0:64, SQT * hb:SQT * (hb + 1)],
                        rb[:])
                at_tiles[p] = at_

        def emit_C(j):
            """Out projection + partial-store + pairwise RS for sq tile j."""
            for c_ in range(4):
                po = poep.tile([128, HID], BF16, tag="po")
                for o in range(2):
                    ps = pp.tile([128, SQT], F32, tag="pq")
                    for dch in range(4):
                        nc.tensor.matmul(
                            ps[:], at_tiles[dch][:, 128 * c_:128 * (c_ + 1)],
                            wot[dch][:, SQT * o:SQT * (o + 1)],
                            start=(dch == 0), stop=(dch == 3))
                    nc.vector.tensor_copy(po[:, SQT * o:SQT * (o + 1)], ps[:])
                r0 = SQT * j + 128 * c_
                nc.sync.dma_start(po_d[r0:r0 + 128, :], po[:])
                if c_ in (1, 3):
                    h0 = SQT * j + 256 * (c_ // 2)
                    k = 2 * j + c_ // 2
                    nc.gpsimd.collective_compute(
                        "ReduceScatter",
                        OP.add,
                        replica_groups=[[0, 1], [2, 3], [4, 5], [6, 7]],
                        ins=[po_d[h0:h0 + 256, :]],
                        outs=[rs_d[128 * k:128 * (k + 1), :]],
                    )

        def emit_ln(c_):
            """Residual + LayerNorm for output chunk c_ (128 rows)."""
            rsb = lp.tile([128, HID], BF16, tag="rsb")
            nc.sync.dma_start(rsb[:], rs_d[128 * c_:128 * (c_ + 1), :])
            xc = lp.tile([128, HID], F32, tag="xc")
            nc.sync.dma_start(xc[:], xh[128 * c_:128 * (c_ + 1), :])
            nc.vector.tensor_add(xc[:], rsb[:], xc[:])
            st6 = lsp.tile([128, 12], F32, tag="st6")
            nc.vector.bn_stats(st6[:, 0:6], xc[:, 0:512])
            nc.vector.bn_stats(st6[:, 6:12], xc[:, 512:1024])
            mv = lsp.tile([128, 2], F32, tag="mv")
            nc.vector.bn_aggr(mv[:], st6[:])
            # 1/sigma via the integer fast-rsqrt seed + 2 Newton steps,
            # entirely on the vector engine: the scalar engine then only
            # ever runs Exp, so its activation table is loaded once.
            ve = lsp.tile([128, 1], F32, tag="ve")
            nc.vector.tensor_scalar_add(ve[:], mv[:, 1:2], EPS)
            inv = lsp.tile([128, 1], F32, tag="inv")
            nc.vector.tensor_scalar(
                inv[:].bitcast(mybir.dt.uint32), ve[:].bitcast(mybir.dt.uint32),
                1, None, op0=OP.logical_shift_right)
            nc.vector.tensor_tensor(
                inv[:].bitcast(mybir.dt.uint32), magicc[:],
                inv[:].bitcast(mybir.dt.uint32), op=OP.subtract)
            nt = lsp.tile([128, 1], F32, tag="nt")
            for _ in range(2):
                nc.vector.tensor_mul(nt[:], inv[:], inv[:])
                nc.vector.scalar_tensor_tensor(
                    nt[:], nt[:], ve[:], mhalf[:], op0=OP.mult, op1=OP.mult)
                nc.vector.tensor_scalar_add(nt[:], nt[:], 1.5)
                nc.vector.tensor_mul(inv[:], inv[:], nt[:])
            nc.vector.scalar_tensor_tensor(
                xc[:], xc[:], mv[:, 0:1], gms[:],
                op0=OP.subtract, op1=OP.mult)
            nc.vector.scalar_tensor_tensor(
                xc[:], xc[:], inv[:], bts[:],
                op0=OP.mult, op1=OP.add)
            nc.sync.dma_start(out[128 * c_:128 * (c_ + 1), :], xc[:])

        # ---- main schedule ----
        emit_A(0)
        for t in range(NSQT):
            def hook(t=t):
                emit_ln(2 * (t - 1))
                emit_ln(2 * (t - 1) + 1)
            emit_B(t, mid_hook=hook if t >= 1 else None)
            if t < NSQT - 1:
                emit_A(t + 1)     # fills normalize latency with proj matmuls
            emit_C(t)
        emit_ln(2 * (NSQT - 1))
        emit_ln(2 * (NSQT - 1) + 1)

    nc.compile()
    return nc


def _prep_inputs(x, Wq, bq, Wk, bk, Wv, bv, Wo, bo, gamma, beta):
    """Shard + lay out the full inputs for the 8 cores."""
    f32 = np.float32
    x = np.asarray(x, f32)
    Wq, bq = np.asarray(Wq, f32), np.asarray(bq, f32)
    Wk, bk = np.asarray(Wk, f32), np.asarray(bk, f32)
    Wv, bv = np.asarray(Wv, f32), np.asarray(bv, f32)
    Wo, bo = np.asarray(Wo, f32), np.asarray(bo, f32)
    gamma, beta = np.asarray(gamma, f32), np.asarray(beta, f32)

    mask = np.triu(np.ones((128, 128), f32))
    vone = np.ones((128, 8), f32)
    gmb = np.ascontiguousarray(np.broadcast_to(gamma, (128, HID)))
    btb = np.ascontiguousarray(np.broadcast_to(beta, (128, HID)))

    halves = []
    for h in range(2):
        sl = slice(HW * h, HW * (h + 1))
        halves.append(dict(
            wqT=np.ascontiguousarray(Wq.T[:, sl]),
            wkT=np.ascontiguousarray(Wk.T[:, sl]),
            wvT=np.ascontiguousarray(Wv.T[:, sl]),
            woT=np.ascontiguousarray(Wo[:, sl].T),
            bq4=np.ascontiguousarray(bq[sl].reshape(4, 128).T),
            bk4=np.ascontiguousarray(bk[sl].reshape(4, 128).T),
            bvb=np.ascontiguousarray(np.broadcast_to(bv[sl], (128, HW))),
        ))

    in_maps = []
    for c in range(N_CORES):
        b, h = c // 2, c % 2
        m = dict(halves[h])
        m["xT"] = np.ascontiguousarray(x[b].T)
        # rows this core receives from the chunked pairwise RS:
        # chunk j covers global rows [512j + 256h, 512j + 256h + 256)
        m["xh"] = np.ascontiguousarray(
            np.concatenate([x[b, 256 * k + 128 * h:256 * k + 128 * h + 128, :]
                            for k in range(8)], axis=0) + bo)
        m["gmb"] = gmb
        m["btb"] = btb
        m["m128"] = mask
        m["vone"] = vone
        in_maps.append(m)
    return in_maps


def _run(inputs, trace=False):
    if "nc" not in _CACHE:
        _CACHE["nc"] = _build()
    nc = _CACHE["nc"]
    in_maps = _prep_inputs(**inputs)
    res = run_bass_kernel_spmd(nc, in_maps, list(range(N_CORES)),
                               trace=trace)
    out = np.empty((B, S, HID), np.float32)
    for c in range(N_CORES):
        b, h = c // 2, c % 2
        o = res.results[c]["out"]
        for k in range(8):
            out[b, 256 * k + 128 * h:256 * k + 128 * h + 128, :] = \
                o[128 * k:128 * (k + 1), :]
    return out, res


def kernel(**inputs):
    out, _ = _run(inputs, trace=False)
    return out

